# revision 2
# baseline (speedup 1.0000x reference)
"""RWKV ChannelMixer (single-token) on 8 Trainium2 NeuronCores.

Math (reference):
    xn  = LayerNorm(x) * ln_w + ln_b
    xk  = xn*tmk + prev*(1-tmk);  xr = xn*tmr + prev*(1-tmr)
    r   = sigmoid(rw @ xr)                       # (D,)
    k   = relu(kw @ xk)^2                        # (F,)
    out = x + r * (vw @ k)                       # (D,)
    returns (out, xn)

Sharding (8 cores):
    kw: F-sharded (512 rows/core) -> local k chunk -> AllGather (2KB/rank)
    vw: D-sharded (128 out rows/core), rw: D-sharded (128 out rows/core)
    LN/mix replicated.  Host only slices inputs / concatenates outputs.

TensorE computes lhsT.T @ rhs contracting over partitions, so the host
pre-transposes and tile-packs each weight into [128, ntiles*128] arrays
whose row order matches the on-chip layout of the activation vectors
(free-major: element d sits at [p=d//8... ] -- see _prep_core below).
"""

import sys
import numpy as np

for _p in ("/opt/trn_rl_repo", "/root/.axon_site/_ro/trn_rl_repo"):
    if _p not in sys.path:
        sys.path.append(_p)

D = 1024
F = 4096
N_CORES = 8
FSH = F // N_CORES      # 512 kw rows per core
DSH = D // N_CORES      # 128 vw/rw output rows per core
LN_EPS = 1e-5

_STATE = {}


def _build():
    import concourse.bacc as bacc
    import concourse.tile as tile
    from concourse import mybir

    f32 = mybir.dt.float32
    Alu = mybir.AluOpType
    Act = mybir.ActivationFunctionType

    nc = bacc.Bacc("TRN2", target_bir_lowering=False, debug=False,
                   num_devices=N_CORES)

    # Per-core DRAM inputs (host pre-packed layouts).
    kw_d = nc.dram_tensor("kw_p", [128, 32 * 128], f32, kind="ExternalInput").ap()
    vw_d = nc.dram_tensor("vw_p", [128, 32 * 128], f32, kind="ExternalInput").ap()
    rw_d = nc.dram_tensor("rw_p", [128, 8 * 128], f32, kind="ExternalInput").ap()
    xv_d = nc.dram_tensor("xv", [128, 8], f32, kind="ExternalInput").ap()
    pv_d = nc.dram_tensor("prev", [128, 8], f32, kind="ExternalInput").ap()
    tk_d = nc.dram_tensor("tmk", [128, 8], f32, kind="ExternalInput").ap()
    tr_d = nc.dram_tensor("tmr", [128, 8], f32, kind="ExternalInput").ap()
    lw_d = nc.dram_tensor("lnw", [128, 8], f32, kind="ExternalInput").ap()
    lb_d = nc.dram_tensor("lnb", [128, 8], f32, kind="ExternalInput").ap()
    xs_d = nc.dram_tensor("xsl", [128, 1], f32, kind="ExternalInput").ap()

    out_d = nc.dram_tensor("out_slice", [128, 1], f32, kind="ExternalOutput").ap()
    xn_d = nc.dram_tensor("xn_out", [128, 8], f32, kind="ExternalOutput").ap()

    with tile.TileContext(nc) as tc:
        with tc.tile_pool(name="w", bufs=1) as wp, \
             tc.tile_pool(name="v", bufs=1) as vp, \
             tc.tile_pool(name="ps", bufs=1, space="PSUM") as pp, \
             tc.tile_pool(name="dr", bufs=1, space="DRAM") as dp:

            # ---- bulk weight DMAs (HWDGE / SP ring, FIFO: kw -> rw -> vw)
            kw_sb = [wp.tile([128, 1024], f32, tag=f"kw{c}", name=f"kw_sb{c}") for c in range(4)]
            rw_sb = wp.tile([128, 1024], f32, tag="rw")
            vw_sb = [wp.tile([128, 1024], f32, tag=f"vw{c}", name=f"vw_sb{c}") for c in range(4)]
            for c in range(4):
                nc.sync.dma_start(out=kw_sb[c][:], in_=kw_d[:, c * 1024:(c + 1) * 1024])
            nc.sync.dma_start(out=rw_sb[:], in_=rw_d[:])
            for c in range(4):
                nc.sync.dma_start(out=vw_sb[c][:], in_=vw_d[:, c * 1024:(c + 1) * 1024])

            # ---- small vector DMAs (SWDGE, concurrent with bulk)
            x_sb = vp.tile([128, 8], f32, tag="x")
            pv_sb = vp.tile([128, 8], f32, tag="pv")
            tk_sb = vp.tile([128, 8], f32, tag="tk")
            tr_sb = vp.tile([128, 8], f32, tag="tr")
            lw_sb = vp.tile([128, 8], f32, tag="lw")
            lb_sb = vp.tile([128, 8], f32, tag="lb")
            xs_sb = vp.tile([128, 1], f32, tag="xs")
            for t, d in ((x_sb, xv_d), (pv_sb, pv_d), (tk_sb, tk_d),
                         (tr_sb, tr_d), (lw_sb, lw_d), (lb_sb, lb_d),
                         (xs_sb, xs_d)):
                nc.gpsimd.dma_start(out=t[:], in_=d[:])

            # ---- constants
            ones_c = vp.tile([128, 1], f32, tag="ones_c")
            ones_r = vp.tile([1, 128], f32, tag="ones_r")
            eps_t = vp.tile([1, 1], f32, tag="eps")
            nc.vector.memset(ones_c[:], 1.0)
            nc.vector.memset(ones_r[:], 1.0)
            nc.vector.memset(eps_t[:], LN_EPS)

            # ---- LayerNorm over all 1024 elements (spread [128, 8])
            s2 = vp.tile([128, 2], f32, tag="s2")       # [sum_x, sum_x2] per part
            xsq = vp.tile([128, 8], f32, tag="xsq")
            nc.vector.tensor_reduce(out=s2[:, 0:1], in_=x_sb[:], axis=mybir.AxisListType.X, op=Alu.add)
            nc.scalar.square(xsq[:], x_sb[:])
            nc.vector.tensor_reduce(out=s2[:, 1:2], in_=xsq[:], axis=mybir.AxisListType.X, op=Alu.add)

            psum_s = pp.tile([1, 2], f32, tag="psum_s")
            nc.tensor.matmul(psum_s[:], ones_c[:], s2[:], start=True, stop=True)
            ssum = vp.tile([1, 2], f32, tag="ssum")     # [mean, E[x^2]]
            nc.scalar.mul(ssum[:], psum_s[:], 1.0 / D)

            mr = vp.tile([1, 2], f32, tag="mr")         # [mean, rstd]
            var_t = vp.tile([1, 1], f32, tag="var")
            std_t = vp.tile([1, 1], f32, tag="std")
            nc.vector.tensor_tensor(mr[:, 0:1], ssum[:, 0:1], ssum[:, 0:1], Alu.mult)  # mean^2
            nc.vector.tensor_tensor(var_t[:], ssum[:, 1:2], mr[:, 0:1], Alu.subtract)  # var
            nc.scalar.activation(std_t[:], var_t[:], Act.Sqrt, bias=eps_t[:])          # sqrt(var+eps)
            nc.vector.reciprocal(mr[:, 1:2], std_t[:])                                 # rstd
            nc.scalar.copy(mr[:, 0:1], ssum[:, 0:1])                                   # mean

            psum_b = pp.tile([128, 2], f32, tag="psum_b")
            nc.tensor.matmul(psum_b[:], ones_r[:], mr[:], start=True, stop=True)
            bc = vp.tile([128, 2], f32, tag="bc")       # broadcast [mean, rstd]
            nc.scalar.copy(bc[:], psum_b[:])

            xn_sb = vp.tile([128, 8], f32, tag="xn")
            nc.vector.tensor_scalar(out=xn_sb[:], in0=x_sb[:],
                                    scalar1=bc[:, 0:1], scalar2=bc[:, 1:2],
                                    op0=Alu.subtract, op1=Alu.mult)
            nc.vector.tensor_mul(xn_sb[:], xn_sb[:], lw_sb[:])
            nc.vector.tensor_add(xn_sb[:], xn_sb[:], lb_sb[:])
            nc.gpsimd.dma_start(out=xn_d[:], in_=xn_sb[:])

            # ---- token mixes: v = prev + tm*(xn - prev)
            xk_sb = vp.tile([128, 8], f32, tag="xk")
            xr_sb = vp.tile([128, 8], f32, tag="xr")
            dmix = vp.tile([128, 8], f32, tag="dmix")
            nc.vector.tensor_sub(dmix[:], xn_sb[:], pv_sb[:])
            nc.vector.tensor_mul(xk_sb[:], dmix[:], tk_sb[:])
            nc.vector.tensor_add(xk_sb[:], xk_sb[:], pv_sb[:])
            nc.vector.tensor_mul(xr_sb[:], dmix[:], tr_sb[:])
            nc.vector.tensor_add(xr_sb[:], xr_sb[:], pv_sb[:])

            # ---- stage A: k_local = sqrelu(kw_i @ xk)  (4 f-chunks x 8 d-tiles)
            psum_k = pp.tile([128, 4], f32, tag="psum_k")
            for c in range(4):
                for j in range(8):
                    nc.tensor.matmul(psum_k[:, c:c + 1],
                                     kw_sb[c][:, j * 128:(j + 1) * 128],
                                     xk_sb[:, j:j + 1],
                                     start=(j == 0), stop=(j == 7))
            krelu = vp.tile([128, 4], f32, tag="krelu")
            k_sb = vp.tile([128, 4], f32, tag="k")
            nc.scalar.activation(krelu[:], psum_k[:], Act.Relu)
            nc.scalar.square(k_sb[:], krelu[:])

            # ---- AllGather k (512 f32/rank -> 4096)
            cc_in = dp.tile([128, 4], f32, tag="cc_in")
            cc_out = dp.tile([4096], f32, tag="cc_out")
            nc.gpsimd.dma_start(out=cc_in[:], in_=k_sb[:])
            nc.gpsimd.collective_compute(
                "AllGather", Alu.bypass,
                replica_groups=[list(range(N_CORES))],
                ins=[cc_in.opt()], outs=[cc_out.opt()],
            )
            k2_sb = vp.tile([128, 32], f32, tag="k2")
            nc.gpsimd.dma_start(out=k2_sb[:], in_=cc_out[:].rearrange("(p j) -> p j", p=128))

            # ---- stage R during the AllGather window: r = sigmoid(rw_i @ xr)
            psum_r = pp.tile([128, 1], f32, tag="psum_r")
            for j in range(8):
                nc.tensor.matmul(psum_r[:],
                                 rw_sb[:, j * 128:(j + 1) * 128],
                                 xr_sb[:, j:j + 1],
                                 start=(j == 0), stop=(j == 7))
            r_sb = vp.tile([128, 1], f32, tag="r")
            nc.scalar.activation(r_sb[:], psum_r[:], Act.Sigmoid)

            # ---- stage V: v_i = vw_i @ k  (32 f-tiles)
            psum_v = pp.tile([128, 1], f32, tag="psum_v")
            for j in range(32):
                nc.tensor.matmul(psum_v[:],
                                 vw_sb[j // 8][:, (j % 8) * 128:(j % 8 + 1) * 128],
                                 k2_sb[:, j:j + 1],
                                 start=(j == 0), stop=(j == 31))

            # ---- out_i = x_i + r * v
            rv = vp.tile([128, 1], f32, tag="rv")
            out_sb = vp.tile([128, 1], f32, tag="out")
            nc.vector.tensor_mul(rv[:], r_sb[:], psum_v[:])
            nc.vector.tensor_add(out_sb[:], rv[:], xs_sb[:])
            nc.gpsimd.dma_start(out=out_d[:], in_=out_sb[:])

    nc.compile()
    return nc


def _prep_shared(kw, vw, rw):
    """Host-side layout prep: transpose + permute + tile-pack the weights.

    Activation layout on chip is free-major: vector element d lives at
    SBUF [p, j] with d = 8*p + j.  TensorE tile for contraction sub-range
    j needs stationary rows ordered by partition p, i.e. row (tile j,
    part p) = global index 8*p + j.
    """
    kwT = np.ascontiguousarray(kw.T)        # (D, F)
    vwT = np.ascontiguousarray(vw.T)        # (F, D)
    rwT = np.ascontiguousarray(rw.T)        # (D, D)

    # permutation of the AllGathered k vector: storage index q -> global f
    q = np.arange(F)
    f_of_q = 512 * (q // 512) + 128 * ((q % 512) % 4) + (q % 512) // 4

    kw_p, vw_p, rw_p = [], [], []
    for i in range(N_CORES):
        # kw: columns (f) 512i..512i+512; rows d = 8p+j -> tiles t = c*8+j
        A = kwT[:, i * FSH:(i + 1) * FSH]           # (1024, 512)
        A = A.reshape(128, 8, 4, 128)               # [p, j, c, m]
        A = A.transpose(0, 2, 1, 3).reshape(128, 4096)  # free idx = (c*8+j)*128+m
        kw_p.append(np.ascontiguousarray(A))

        # rw: columns (d out) 128i..128i+128; rows d = 8p+j -> tiles j
        B = rwT[:, i * DSH:(i + 1) * DSH]           # (1024, 128)
        B = B.reshape(128, 8, 128).reshape(128, 1024)   # [p, j*128+m]
        rw_p.append(np.ascontiguousarray(B))

        # vw: columns (d out) 128i..128i+128; rows f in AllGather order:
        # tile j, part p  <- global f = f_of_q(32p + j)
        C = vwT[:, i * DSH:(i + 1) * DSH]           # (4096, 128)
        C = C[f_of_q]                               # row q -> f_of_q(q)
        C = C.reshape(128, 32, 128)                 # q = 32p + j -> [p, j, m]
        vw_p.append(np.ascontiguousarray(C.reshape(128, 4096)))
    return kw_p, vw_p, rw_p


def kernel(x, state, time_mix_k, time_mix_r, kw, vw, rw, ln_weight, ln_bias):
    from concourse import bass_utils

    x = np.asarray(x, dtype=np.float32)
    state = np.asarray(state, dtype=np.float32)
    kw = np.asarray(kw, dtype=np.float32)
    vw = np.asarray(vw, dtype=np.float32)
    rw = np.asarray(rw, dtype=np.float32)
    tmk = np.asarray(time_mix_k, dtype=np.float32)
    tmr = np.asarray(time_mix_r, dtype=np.float32)
    lnw = np.asarray(ln_weight, dtype=np.float32)
    lnb = np.asarray(ln_bias, dtype=np.float32)

    if "nc" not in _STATE:
        _STATE["nc"] = _build()
    nc = _STATE["nc"]

    kw_p, vw_p, rw_p = _prep_shared(kw, vw, rw)

    xv = np.ascontiguousarray(x.reshape(128, 8))
    pv = np.ascontiguousarray(state[0].reshape(128, 8))
    tk = np.ascontiguousarray(tmk.reshape(128, 8))
    tr = np.ascontiguousarray(tmr.reshape(128, 8))
    lw = np.ascontiguousarray(lnw.reshape(128, 8))
    lb = np.ascontiguousarray(lnb.reshape(128, 8))

    in_maps = []
    for i in range(N_CORES):
        in_maps.append({
            "kw_p": kw_p[i], "vw_p": vw_p[i], "rw_p": rw_p[i],
            "xv": xv, "prev": pv, "tmk": tk, "tmr": tr,
            "lnw": lw, "lnb": lb,
            "xsl": np.ascontiguousarray(x[i * DSH:(i + 1) * DSH].reshape(128, 1)),
        })

    res = bass_utils.run_bass_kernel_spmd(nc, in_maps, core_ids=list(range(N_CORES)))

    out = np.concatenate([res.results[i]["out_slice"].reshape(DSH)
                          for i in range(N_CORES)])
    xn = res.results[0]["xn_out"].reshape(D)
    return np.asarray(out, dtype=np.float32), np.asarray(xn, dtype=np.float32)


# revision 10
# speedup vs baseline: 2.6318x; 2.6318x over previous
"""RWKV ChannelMixer (single-token) on 8 Trainium2 NeuronCores.

Math (reference):
    xn  = LayerNorm(x) * ln_w + ln_b
    xk  = xn*tmk + prev*(1-tmk);  xr = xn*tmr + prev*(1-tmr)
    r   = sigmoid(rw @ xr)                       # (D,)
    k   = relu(kw @ xk)^2                        # (F,)
    out = x + r * (vw @ k)                       # (D,)
    returns (out, xn)

Sharding (8 cores, no collectives -- cross-core sync costs ~60us here):
    kw: F-row-sharded (512 rows/core)  -> local k chunk (512,)
    vw: F-col-sharded (512 cols/core)  -> partial v_i = vw[:,Fi] @ k_i (1024,)
    rw: D-row-sharded (128 rows/core)  -> r chunk (128,)
    LN/mix replicated.  Host unshard: v = sum_i v_i, r = concat(r_i),
    out = x + r*v.

Engines: dot-products run on the Vector engine (tensor_tensor_reduce,
fp32 @ ~1 elem/lane/cycle ~ 490GB/s > 358GB/s HBM/core, so the kernel
stays DMA-bound).  TensorE only does tiny selector-matmul broadcasts /
transposes.  Weight matrices stream through SBUF in natural row-major
layout (host reshapes rows onto 128 partitions; no transposes).
"""

import sys
import numpy as np

for _p in ("/opt/trn_rl_repo", "/root/.axon_site/_ro/trn_rl_repo"):
    if _p not in sys.path:
        sys.path.append(_p)

D = 1024
F = 4096
N_CORES = 8
FSH = F // N_CORES      # 512 kw rows / vw cols per core
DSH = D // N_CORES      # 128 rw rows per core
LN_EPS = 1e-5

_STATE = {}


def _body(nc, tc, mybir, stage):
    f32 = mybir.dt.float32
    Alu = mybir.AluOpType
    Act = mybir.ActivationFunctionType
    AxX = mybir.AxisListType.X

    kw_d = nc.dram_tensor("kw_p", [128, 4096], f32, kind="ExternalInput").ap()
    vw_d = nc.dram_tensor("vw_p", [128, 4096], f32, kind="ExternalInput").ap()
    rw_d = nc.dram_tensor("rw_p", [128, 1024], f32, kind="ExternalInput").ap()
    # x, prev, tmk, tmr, lnw, lnb stacked: [8, 6*128], row j = vectors' d-slice j
    sm_d = nc.dram_tensor("smalls", [8, 768], f32, kind="ExternalInput").ap()

    xn_d = nc.dram_tensor("xn_out", [8, 128], f32, kind="ExternalOutput").ap()
    v_d = nc.dram_tensor("v_out", [8, 128], f32, kind="ExternalOutput").ap()
    r_d = nc.dram_tensor("r_out", [1, 128], f32, kind="ExternalOutput").ap()

    import contextlib
    with contextlib.ExitStack() as ctx:
        wp = ctx.enter_context(tc.tile_pool(name="w", bufs=1))
        vp = ctx.enter_context(tc.tile_pool(name="v", bufs=1))
        bp = ctx.enter_context(tc.tile_pool(name="bc", bufs=2, space="PSUM"))
        pp = ctx.enter_context(tc.tile_pool(name="ps", bufs=1, space="PSUM"))

        # ---- small packed DMA first (SWDGE), then bulk (HWDGE FIFO kw->rw->vw)
        sm_sb = vp.tile([8, 768], f32, tag="sm")
        nc.gpsimd.dma_start(out=sm_sb[:], in_=sm_d[:])
        x_row = sm_sb[:, 0:128]
        pv_row = sm_sb[:, 128:256]
        tk_row = sm_sb[:, 256:384]
        tr_row = sm_sb[:, 384:512]
        lw_row = sm_sb[:, 512:640]
        lb_row = sm_sb[:, 640:768]

        if stage >= 2:
            kw_sb = wp.tile([128, 4096], f32, tag="kw")
            rw_sb = wp.tile([128, 1024], f32, tag="rw")
            vw_sb = wp.tile([128, 4096], f32, tag="vw")
            for c in range(4):
                nc.sync.dma_start(out=kw_sb[:, c * 1024:(c + 1) * 1024],
                                  in_=kw_d[:, c * 1024:(c + 1) * 1024])
            nc.sync.dma_start(out=rw_sb[:], in_=rw_d[:])
            for c in range(4):
                nc.sync.dma_start(out=vw_sb[:, c * 1024:(c + 1) * 1024],
                                  in_=vw_d[:, c * 1024:(c + 1) * 1024])

        # ---- constants
        ones_c8 = vp.tile([8, 1], f32, tag="ones_c8")
        ones_r8 = vp.tile([1, 8], f32, tag="ones_r8")
        eps_t = vp.tile([1, 1], f32, tag="eps")
        nc.vector.memset(ones_c8[:], 1.0)
        nc.vector.memset(ones_r8[:], 1.0)
        nc.vector.memset(eps_t[:], LN_EPS)
        if stage >= 3:
            # one-hot row-selector matrices (lhsT for row-broadcast matmuls)
            sel8 = vp.tile([8, 1024], f32, tag="sel8")
            sel4 = vp.tile([4, 512], f32, tag="sel4")
            nc.gpsimd.memset(sel8[:], 0.0)
            nc.gpsimd.memset(sel4[:], 0.0)
            nc.gpsimd.affine_select(
                out=sel8[:].rearrange("p (j q) -> p j q", j=8),
                in_=sel8[:].rearrange("p (j q) -> p j q", j=8),
                compare_op=Alu.not_equal, fill=1.0, base=0,
                pattern=[[-1, 8], [0, 128]], channel_multiplier=1)
            nc.gpsimd.affine_select(
                out=sel4[:].rearrange("p (j q) -> p j q", j=4),
                in_=sel4[:].rearrange("p (j q) -> p j q", j=4),
                compare_op=Alu.not_equal, fill=1.0, base=0,
                pattern=[[-1, 4], [0, 128]], channel_multiplier=1)

        # ---- LayerNorm stats over 1024 elems laid out [8, 128]
        s2 = vp.tile([8, 2], f32, tag="s2")
        xsq = vp.tile([8, 128], f32, tag="xsq")
        nc.vector.tensor_reduce(out=s2[:, 0:1], in_=x_row, axis=AxX, op=Alu.add)
        nc.scalar.activation(xsq[:], x_row, Act.Square, accum_out=s2[:, 1:2])

        psum_s = pp.tile([1, 2], f32, tag="pmisc", bufs=2)
        nc.tensor.matmul(psum_s[:], ones_c8[:], s2[:], start=True, stop=True)
        ssum = vp.tile([1, 2], f32, tag="ssum")     # [mean, E[x^2]]
        nc.scalar.mul(ssum[:], psum_s[:], 1.0 / D)

        mr = vp.tile([1, 2], f32, tag="mr")         # [mean, rstd]
        var_t = vp.tile([1, 1], f32, tag="var")
        std_t = vp.tile([1, 1], f32, tag="std")
        nc.vector.tensor_tensor(mr[:, 0:1], ssum[:, 0:1], ssum[:, 0:1], Alu.mult)
        nc.vector.tensor_tensor(var_t[:], ssum[:, 1:2], mr[:, 0:1], Alu.subtract)
        nc.scalar.activation(std_t[:], var_t[:], Act.Sqrt, bias=eps_t[:])
        nc.vector.reciprocal(mr[:, 1:2], std_t[:])
        nc.scalar.copy(mr[:, 0:1], ssum[:, 0:1])

        psum_b = pp.tile([8, 2], f32, tag="pmisc", bufs=2)
        nc.tensor.matmul(psum_b[:], ones_r8[:], mr[:], start=True, stop=True)
        bc8 = vp.tile([8, 2], f32, tag="bc8")
        nc.scalar.copy(bc8[:], psum_b[:])

        xn_row = vp.tile([8, 128], f32, tag="xn")
        nc.vector.tensor_scalar(out=xn_row[:], in0=x_row,
                                scalar1=bc8[:, 0:1], scalar2=bc8[:, 1:2],
                                op0=Alu.subtract, op1=Alu.mult)
        nc.vector.tensor_mul(xn_row[:], xn_row[:], lw_row)
        nc.vector.tensor_add(xn_row[:], xn_row[:], lb_row)
        nc.gpsimd.dma_start(out=xn_d[:], in_=xn_row[:])

        # ---- token mixes: xk = prev + tmk*(xn-prev), same for xr
        dmix = vp.tile([8, 128], f32, tag="dmix")
        xk_row = vp.tile([8, 128], f32, tag="xk")
        xr_row = vp.tile([8, 128], f32, tag="xr")
        nc.vector.tensor_sub(dmix[:], xn_row[:], pv_row)
        nc.vector.tensor_mul(xk_row[:], dmix[:], tk_row)
        nc.vector.tensor_add(xk_row[:], xk_row[:], pv_row)
        nc.vector.tensor_mul(xr_row[:], dmix[:], tr_row)
        nc.vector.tensor_add(xr_row[:], xr_row[:], pv_row)

        if stage < 3:
            return

        # ---- broadcast xk across partitions: [8,128] -> [128, 1024]
        xk_bc = vp.tile([128, 1024], f32, tag="xk_bc")
        for j in range(8):
            pb = bp.tile([128, 128], f32, tag="pb", name=f"pbk{j}")
            nc.tensor.matmul(pb[:], sel8[:, j * 128:(j + 1) * 128], xk_row[:],
                             start=True, stop=True)
            nc.scalar.copy(xk_bc[:, j * 128:(j + 1) * 128], pb[:])

        if stage < 4:
            return

        # ---- stage A: k chunk = sqrelu(kw_i @ xk); kw tile c = rows 128c..
        scratch = vp.tile([128, 1024], f32, tag="scratch")
        k_sb = vp.tile([128, 4], f32, tag="k")
        for c in range(4):
            nc.vector.scalar_tensor_tensor(
                out=scratch[:], in0=kw_sb[:, c * 1024:(c + 1) * 1024],
                scalar=1.0, in1=xk_bc[:],
                op0=Alu.mult, op1=Alu.mult, accum_out=k_sb[:, c:c + 1])
        krelu = vp.tile([128, 4], f32, tag="krelu")
        ksq = vp.tile([128, 4], f32, tag="ksq")
        nc.scalar.activation(krelu[:], k_sb[:], Act.Relu)
        nc.scalar.square(ksq[:], krelu[:])

        if stage < 5:
            return

        # ---- broadcast xr (during kw dots) and compute r
        xr_bc = vp.tile([128, 1024], f32, tag="xr_bc")
        for j in range(8):
            pb = bp.tile([128, 128], f32, tag="pb", name=f"pbr{j}")
            nc.tensor.matmul(pb[:], sel8[:, j * 128:(j + 1) * 128], xr_row[:],
                             start=True, stop=True)
            nc.scalar.copy(xr_bc[:, j * 128:(j + 1) * 128], pb[:])

        pre_r = vp.tile([128, 1], f32, tag="pre_r")
        nc.vector.scalar_tensor_tensor(
            out=scratch[:], in0=rw_sb[:], scalar=1.0, in1=xr_bc[:],
            op0=Alu.mult, op1=Alu.mult, accum_out=pre_r[:])
        r_sb = vp.tile([128, 1], f32, tag="r")
        nc.scalar.activation(r_sb[:], pre_r[:], Act.Sigmoid)

        if stage < 6:
            return

        # ---- k broadcast: transpose [128,4] -> [4,128], then 4 row-bcasts
        from concourse.masks import make_identity
        ident = vp.tile([128, 128], f32, tag="ident")
        make_identity(nc, ident)
        kT_ps = pp.tile([4, 128], f32, tag="pmisc", bufs=2)
        nc.tensor.transpose(kT_ps[:], ksq[:], ident[:])
        kT = vp.tile([4, 128], f32, tag="kT")
        nc.scalar.copy(kT[:], kT_ps[:])
        k_bc = vp.tile([128, 512], f32, tag="k_bc")
        for c in range(4):
            pb = bp.tile([128, 128], f32, tag="pb", name=f"pbc{c}")
            nc.tensor.matmul(pb[:], sel4[:, c * 128:(c + 1) * 128], kT[:],
                             start=True, stop=True)
            nc.scalar.copy(k_bc[:, c * 128:(c + 1) * 128], pb[:])

        # ---- stage V: v partial, 8 d-chunks of [128, 512] x k_bc
        v_sb = vp.tile([128, 8], f32, tag="v")
        for m in range(8):
            nc.vector.scalar_tensor_tensor(
                out=scratch[:, 0:512], in0=vw_sb[:, m * 512:(m + 1) * 512],
                scalar=1.0, in1=k_bc[:],
                op0=Alu.mult, op1=Alu.mult, accum_out=v_sb[:, m:m + 1])

        # ---- outputs in row form (contiguous DMA): transpose via PE
        vT_ps = pp.tile([8, 128], f32, tag="pmisc", bufs=2)
        nc.tensor.transpose(vT_ps[:], v_sb[:], ident[:])
        vT = vp.tile([8, 128], f32, tag="vT")
        nc.scalar.copy(vT[:], vT_ps[:])
        nc.gpsimd.dma_start(out=v_d[:], in_=vT[:])

        rT_ps = pp.tile([1, 128], f32, tag="pmisc", bufs=2)
        nc.tensor.transpose(rT_ps[:], r_sb[:], ident[:])
        rT = vp.tile([1, 128], f32, tag="rT")
        nc.scalar.copy(rT[:], rT_ps[:])
        nc.gpsimd.dma_start(out=r_d[:], in_=rT[:])


def _build(stage=6):
    import concourse.bacc as bacc
    import concourse.tile as tile
    from concourse import mybir

    nc = bacc.Bacc("TRN2", target_bir_lowering=False, debug=False,
                   num_devices=N_CORES)
    with tile.TileContext(nc) as tc:
        _body(nc, tc, mybir, stage)
    nc.compile()
    return nc


def _prep_shared(kw, vw, rw):
    """Slice + reshape weights per core (rows onto 128 partitions)."""
    kw_p, vw_p, rw_p = [], [], []
    for i in range(N_CORES):
        A = kw[i * FSH:(i + 1) * FSH, :]                # (512, 1024) rows f
        A = A.reshape(4, 128, 1024).transpose(1, 0, 2)  # [p, c, d]
        kw_p.append(np.ascontiguousarray(A.reshape(128, 4096)))

        B = rw[i * DSH:(i + 1) * DSH, :]                # (128, 1024) rows d
        rw_p.append(np.ascontiguousarray(B))

        C = vw[:, i * FSH:(i + 1) * FSH]                # (1024, 512) rows d
        C = C.reshape(8, 128, FSH).transpose(1, 0, 2)   # [p, m, f]
        vw_p.append(np.ascontiguousarray(C.reshape(128, 4096)))
    return kw_p, vw_p, rw_p


def _prep_smalls(x, state, tmk, tmr, lnw, lnb):
    sm = np.stack([x.reshape(8, 128), state[0].reshape(8, 128),
                   tmk.reshape(8, 128), tmr.reshape(8, 128),
                   lnw.reshape(8, 128), lnb.reshape(8, 128)], axis=1)
    return np.ascontiguousarray(sm.reshape(8, 768))


def kernel(x, state, time_mix_k, time_mix_r, kw, vw, rw, ln_weight, ln_bias):
    from concourse import bass_utils

    x = np.asarray(x, dtype=np.float32)
    state = np.asarray(state, dtype=np.float32)
    kw = np.asarray(kw, dtype=np.float32)
    vw = np.asarray(vw, dtype=np.float32)
    rw = np.asarray(rw, dtype=np.float32)
    tmk = np.asarray(time_mix_k, dtype=np.float32)
    tmr = np.asarray(time_mix_r, dtype=np.float32)
    lnw = np.asarray(ln_weight, dtype=np.float32)
    lnb = np.asarray(ln_bias, dtype=np.float32)

    if "nc" not in _STATE:
        _STATE["nc"] = _build()
    nc = _STATE["nc"]

    kw_p, vw_p, rw_p = _prep_shared(kw, vw, rw)
    sm = _prep_smalls(x, state, tmk, tmr, lnw, lnb)

    in_maps = [{"kw_p": kw_p[i], "vw_p": vw_p[i], "rw_p": rw_p[i], "smalls": sm}
               for i in range(N_CORES)]

    res = bass_utils.run_bass_kernel_spmd(nc, in_maps, core_ids=list(range(N_CORES)))

    # unshard: v = sum of partials, r = concat of chunks
    v = np.zeros(D, dtype=np.float64)
    for i in range(N_CORES):
        v += res.results[i]["v_out"].reshape(D).astype(np.float64)
    r = np.concatenate([res.results[i]["r_out"].reshape(DSH)
                        for i in range(N_CORES)])
    out = x + r * v.astype(np.float32)
    xn = res.results[0]["xn_out"].reshape(D)
    return np.asarray(out, dtype=np.float32), np.asarray(xn, dtype=np.float32)
